# revision 1
# baseline (speedup 1.0000x reference)
"""Trainium2 Bass kernel for a dense transformer block.

Block: split-LayerNorm -> attention -> residual -> split-LayerNorm -> MLP(GELU)
-> residual.  Shapes: B=8, N=1024, D=768, H=12 heads (hd=64), HID=3072.

Sharding: pure data-parallel over batch -- one batch element per NeuronCore
(8 cores), all weights replicated, no collectives.

On-chip layout is feature-major (activations stored transposed, [feature, token])
so every matmul consumes activations as the moving operand directly and
LayerNorm/softmax cross-feature reductions map onto PE matmuls with
indicator/ones vectors.  The host pre-transposes x and all weight matrices so
every DMA is contiguous.
"""

import os
import numpy as np
import ml_dtypes

import concourse.bass as bass
import concourse.tile as tile
from concourse import bacc, mybir
from concourse.bass_utils import run_bass_kernel_spmd
from contextlib import ExitStack

F32 = mybir.dt.float32
BF16 = mybir.dt.bfloat16
F32R = mybir.dt.float32r
AF = mybir.ActivationFunctionType
ALU = mybir.AluOpType

D = 768
H = 12
HD = 64
HID = 3072
NT = 1024  # tokens per core
B = 8
S1 = 320  # split-LN segment boundaries: [0,320), [320,384), [384,768)
S2 = 384
SCALE = 0.125  # (D//H) ** -0.5 = 64 ** -0.5
EPS = 1e-5
P = 128

KC = D // P        # 6  c-chunks
MQKV = 3 * D // P  # 18 qkv out chunks
MFC1 = HID // P    # 24


def _halves():
    return (slice(0, 512), slice(512, 1024))


# Per segment: (ind_bT column range for the broadcast lhsT,
#               [(chunk, row0, row1), ...] applied regions)
LN_REGIONS = [
    (slice(0, P), [(0, 0, P), (1, 0, P), (2, 0, HD)]),        # seg0 [0,320)
    (slice(S1, S2), [(2, HD, P)]),                            # seg1 [320,384)
    (slice(S2, S2 + P), [(3, 0, P), (4, 0, P), (5, 0, P)]),   # seg2 [384,768)
]


def _layernorm(tc, xat, g_tile, b_tile, ind_sum_tiles, ind_bT, invlen,
               eps_t, psmm, out_pool, tag):
    """Split-LayerNorm over the feature dim (partitions).  xat(k, hi)
    returns the [128, 512] fp32 feature-major input slab for chunk k,
    token-half hi.  Returns dict[(k, hi)] of [128, 512] bf16 tiles."""
    nc = tc.nc
    with tc.tile_pool(name=f"ln_{tag}", bufs=3) as lnp, \
         tc.tile_pool(name=f"lnt_{tag}", bufs=KC) as lnt, \
         tc.tile_pool(name=f"lns_{tag}", bufs=1) as lns, \
         tc.tile_pool(name=f"lnps_{tag}", bufs=2, space="PSUM") as psstat:
        # segment sums via indicator matmuls: sums[s, q] = sum_{c in seg_s} x[c, q]
        sums_t = psstat.tile([P, NT], F32, tag="stat")
        sumsq_t = psstat.tile([P, NT], F32, tag="stat")
        sums = sums_t[0:3, :]
        sumsq = sumsq_t[0:3, :]
        for k in range(KC):
            for hi, hs in enumerate(_halves()):
                xbk = lnp.tile([P, 512], BF16, tag="xb")
                nc.vector.tensor_copy(xbk[:], xat(k, hi))
                xqk = lnp.tile([P, 512], BF16, tag="xq")
                nc.scalar.activation(xqk[:], xat(k, hi), AF.Square)
                nc.tensor.matmul(sums_t[:, hs], ind_sum_tiles[k][:],
                                 xbk[:],
                                 start=(k == 0), stop=(k == KC - 1))
                nc.tensor.matmul(sumsq_t[:, hs], ind_sum_tiles[k][:],
                                 xqk[:],
                                 start=(k == 0), stop=(k == KC - 1))
        # Stats + broadcast + apply, pipelined per token-half so the first
        # half's normalized chunks unblock downstream matmuls early.
        mean = lns.tile([3, NT], F32, tag="mean")
        nmsq = lns.tile([3, NT], F32, tag="nmsq")
        var = lns.tile([3, NT], F32, tag="var")
        std = lns.tile([3, NT], F32, tag="std")
        rstd = lns.tile([3, NT], F32, tag="rstd")
        scr = lns.tile([3, NT], F32, tag="scr")
        beta = lns.tile([3, NT], F32, tag="beta")
        rstd_r = lns.tile([3, NT], F32R, tag="rstd_r")
        beta_r = lns.tile([3, NT], F32R, tag="beta_r")
        t1s = {}
        t2s = {}
        out_tiles = {}
        for hi, hs in enumerate(_halves()):
            nc.vector.tensor_scalar_mul(mean[:, hs], sums[:, hs], invlen[:])
            nc.vector.scalar_tensor_tensor(nmsq[:, hs], mean[:, hs], -1.0,
                                           mean[:, hs], ALU.mult, ALU.mult)
            nc.vector.scalar_tensor_tensor(var[:, hs], sumsq[:, hs],
                                           invlen[:], nmsq[:, hs],
                                           ALU.mult, ALU.add)
            nc.scalar.activation(std[:, hs], var[:, hs], AF.Sqrt,
                                 bias=eps_t[:])
            nc.vector.reciprocal_approx_accurate(rstd[:, hs], std[:, hs],
                                                 scr[:, hs])
            nc.vector.scalar_tensor_tensor(beta[:, hs], mean[:, hs], -1.0,
                                           rstd[:, hs], ALU.mult, ALU.mult)
            nc.vector.tensor_copy(rstd_r[:, hs], rstd[:, hs])
            nc.vector.tensor_copy(beta_r[:, hs], beta[:, hs])
            for cols, regions in LN_REGIONS:
                m_rows = max(r1 - r0 for _, r0, r1 in regions)
                aB = psmm.tile([P, NT], F32, tag="mm")
                bB = psmm.tile([P, NT], F32, tag="mm")
                nc.tensor.matmul(aB[0:m_rows, hs],
                                 ind_bT[:, cols][:, 0:m_rows],
                                 rstd_r[:, hs], start=True, stop=True)
                nc.tensor.matmul(bB[0:m_rows, hs],
                                 ind_bT[:, cols][:, 0:m_rows],
                                 beta_r[:, hs], start=True, stop=True)
                for k, r0, r1 in regions:
                    if (k, hi) not in t1s:
                        t1k = lnt.tile([P, 512], F32, tag="t1")
                        t2k = lnt.tile([P, 512], F32, tag="t2")
                        t1s[(k, hi)] = t1k
                        t2s[(k, hi)] = t2k
                        nx_k = out_pool.tile([P, 512], BF16,
                                             tag=f"normx_{tag}")
                        out_tiles[(k, hi)] = nx_k
                    gcol = g_tile[r0:r1, k:k + 1]
                    # t1 = (x * g) * rstd_bcast ; t2 = (-mean*rstd*g) + t1
                    nc.vector.scalar_tensor_tensor(
                        t1s[(k, hi)][r0:r1, :], xat(k, hi)[r0:r1, :], gcol,
                        aB[0:r1 - r0, hs], ALU.mult, ALU.mult)
                    nc.vector.scalar_tensor_tensor(
                        t2s[(k, hi)][r0:r1, :], bB[0:r1 - r0, hs], gcol,
                        t1s[(k, hi)][r0:r1, :], ALU.mult, ALU.add)
                    if r1 == P:
                        nc.scalar.activation(out_tiles[(k, hi)][:],
                                             t2s[(k, hi)][:], AF.Identity,
                                             bias=b_tile[:, k:k + 1])
        return out_tiles


DEBUG = bool(int(os.environ.get("KBG_DEBUG", "0")))


def build():
    nc = bacc.Bacc("TRN2", target_bir_lowering=False, debug=False)
    dbg = {}
    if DEBUG:
        dbg["normx"] = nc.dram_tensor("dbg_normx", [D, NT], BF16, kind="ExternalOutput")
        dbg["qkvT"] = nc.dram_tensor("dbg_qkvT", [3 * D, NT], BF16, kind="ExternalOutput")
        dbg["yt"] = nc.dram_tensor("dbg_yt", [D, NT], BF16, kind="ExternalOutput")
        dbg["x1"] = nc.dram_tensor("dbg_x1", [D, NT], F32, kind="ExternalOutput")
        dbg["normx2"] = nc.dram_tensor("dbg_normx2", [D, NT], BF16, kind="ExternalOutput")
        dbg["hT"] = nc.dram_tensor("dbg_hT", [HID, NT], BF16, kind="ExternalOutput")


    xT = nc.dram_tensor("xT", [D, NT], F32, kind="ExternalInput")
    wqkvT = nc.dram_tensor("wqkvT", [D, 3 * D], BF16, kind="ExternalInput")
    wprojT = nc.dram_tensor("wprojT", [D, D], BF16, kind="ExternalInput")
    wfc1T = nc.dram_tensor("wfc1T", [D, HID], BF16, kind="ExternalInput")
    wfc2T = nc.dram_tensor("wfc2T", [HID, D], BF16, kind="ExternalInput")
    pbias = nc.dram_tensor("pbias", [D], F32, kind="ExternalInput")
    fc1b = nc.dram_tensor("fc1b", [HID], F32, kind="ExternalInput")
    fc2b = nc.dram_tensor("fc2b", [D], F32, kind="ExternalInput")
    g1d = nc.dram_tensor("g1", [D], F32, kind="ExternalInput")
    b1d = nc.dram_tensor("b1", [D], F32, kind="ExternalInput")
    g2d = nc.dram_tensor("g2", [D], F32, kind="ExternalInput")
    b2d = nc.dram_tensor("b2", [D], F32, kind="ExternalInput")
    indsum = nc.dram_tensor("indsum", [D, P], BF16, kind="ExternalInput")
    indbTd = nc.dram_tensor("indbT", [3, D], F32R, kind="ExternalInput")
    invlend = nc.dram_tensor("invlen", [3, 1], F32, kind="ExternalInput")
    outT = nc.dram_tensor("outT", [D, NT], F32, kind="ExternalOutput")

    with tile.TileContext(nc) as tc, ExitStack() as ctx:
        const = ctx.enter_context(tc.tile_pool(name="const", bufs=1))
        psmm = ctx.enter_context(tc.tile_pool(name="psmm", bufs=2, space="PSUM"))

        # constants
        eps_t = const.tile([3, 1], F32)
        nc.vector.memset(eps_t[:], EPS)
        ones64 = const.tile([1, HD], F32)
        nc.vector.memset(ones64[:], 1.0)

        def load_cols(dram, n):
            t = const.tile([P, n], F32, tag=f"c_{dram.name}")
            nc.sync.dma_start(t[:], dram.ap().rearrange("(a p) -> p a", p=P))
            return t

        pb = load_cols(pbias, KC)
        f1b = load_cols(fc1b, MFC1)
        f2b = load_cols(fc2b, KC)
        g1 = load_cols(g1d, KC)
        b1 = load_cols(b1d, KC)
        g2 = load_cols(g2d, KC)
        b2 = load_cols(b2d, KC)
        ind_sum_tiles = []
        for k in range(KC):
            t = const.tile([P, P], BF16, tag=f"inds{k}")
            nc.sync.dma_start(t[:], indsum[k * P:(k + 1) * P, :])
            ind_sum_tiles.append(t)
        ind_bT = const.tile([3, D], F32R)
        nc.sync.dma_start(ind_bT[:], indbTd[:])
        invlen = const.tile([3, 1], F32)
        nc.sync.dma_start(invlen[:], invlend[:])

        x1pool = ctx.enter_context(tc.tile_pool(name="x1pool", bufs=2 * KC))

        stage1 = ctx.enter_context(ExitStack())  # x0, lives through proj
        x0pool = stage1.enter_context(tc.tile_pool(name="x0pool", bufs=KC))
        x_tiles = []
        for k in range(KC):
            t = x0pool.tile([P, NT], F32, tag="x0")
            for qr in range(4):
                nc.sync.dma_start(t[qr * 32:(qr + 1) * 32, :],
                                  xT[k * P + qr * 32:k * P + (qr + 1) * 32, :])
            x_tiles.append(t)

        # ---- LN1 ----
        nx_stage = ctx.enter_context(ExitStack())  # normx, lives through qkv
        nx_pool = nx_stage.enter_context(tc.tile_pool(name="nx_pool", bufs=2 * KC))
        normx = _layernorm(tc,
                           lambda k, hi: x_tiles[k][:, _halves()[hi]],
                           g1, b1, ind_sum_tiles, ind_bT,
                           invlen, eps_t, psmm, nx_pool, "ln1")
        if DEBUG:
            for k in range(KC):
                for hi, hs in enumerate(_halves()):
                    nc.sync.dma_start(dbg["normx"][k * P:(k + 1) * P, hs],
                                      normx[(k, hi)][:])

        # ---- fused QKV + attention, one head-pair at a time ----
        # Pair j computes qkv chunks {j, 6+j, 12+j}, builds the pair's V
        # tiles, then runs both heads.  Attention's ACT-heavy exp stream
        # overlaps the next pair's PE-heavy qkv matmuls.
        y_stage = ctx.enter_context(ExitStack())
        y_pool = y_stage.enter_context(
            tc.tile_pool(name="y_pool", bufs=KC, side="right"))
        qkv_stage = ctx.enter_context(ExitStack())
        q_pool = qkv_stage.enter_context(
            tc.tile_pool(name="q_pool", bufs=9, side="right"))
        yt = []
        for _yi in range(KC):
            yt_t = y_pool.tile([P, NT], BF16, tag="yt")
            yt.append(yt_t)
        with tc.tile_pool(name="wqkv", bufs=KC) as wq_pool, \
             tc.tile_pool(name="v_pool", bufs=4) as v_pool, \
             tc.tile_pool(name="e_pool", bufs=4) as e_pool, \
             tc.tile_pool(name="kp_pool", bufs=3) as kp_pool, \
             tc.tile_pool(name="psot", bufs=2, space="PSUM") as psot, \
             tc.tile_pool(name="sm_pool", bufs=3) as sm_pool:
            wq = []
            for k in range(KC):
                t = wq_pool.tile([P, 3 * D], BF16, tag="wqkv")
                nc.sync.dma_start(t[:], wqkvT[k * P:(k + 1) * P, :])
                wq.append(t)

            # Per-head normalization tail, deferred into the next head's
            # loop so the DVE reciprocal latency hides under its matmuls.
            def make_tail(h, ot):
                po = (h % 2) * HD

                def tail():
                    dnm = sm_pool.tile([1, NT], F32, tag="dnm")
                    nc.vector.tensor_copy(dnm[:], ot[64:65, :])
                    r = sm_pool.tile([1, NT], F32, tag="recip")
                    nc.vector.reciprocal_approx_fast(r[:], dnm[:])
                    rbs = sm_pool.tile([HD, NT], F32, tag="rbs")
                    nc.gpsimd.partition_broadcast(rbs[:], r[:])
                    nc.vector.tensor_mul(yt[h // 2][po:po + HD, :],
                                         ot[0:HD, :], rbs[:])
                return tail

            pending_tails = []
            for j in range(KC):
                qkvT_j = {}
                for m in (j, 6 + j, 12 + j):
                    ps = psmm.tile([P, NT], F32, tag="mm")
                    for hi, hs in enumerate(_halves()):
                        for k in range(KC):
                            nc.tensor.matmul(ps[:, hs],
                                             wq[k][:, m * P:(m + 1) * P],
                                             normx[(k, hi)][:],
                                             start=(k == 0), stop=(k == KC - 1))
                    qt = q_pool.tile([P, NT], BF16, tag="qkv")
                    nc.vector.tensor_copy(qt[:], ps[:])
                    qkvT_j[m] = qt
                    if DEBUG:
                        nc.sync.dma_start(dbg["qkvT"][m * P:(m + 1) * P, :],
                                          qt[:])
                # V token-major via DMA transpose (plus the all-ones
                # columns from the memset giving the softmax denominator at
                # output row 64)
                vts = []
                vsl = qkvT_j[12 + j]
                for hh in range(2):
                    po = hh * HD
                    vt = v_pool.tile([P, 8 * P], BF16, tag="vaug")
                    nc.vector.memset(vt[:], 1.0)
                    for kc in range(8):
                        nc.sync.dma_start(
                            vt[:, kc * P:kc * P + HD],
                            vsl[po:po + HD, kc * P:(kc + 1) * P],
                            transpose=True)
                    vts.append(vt)
                for hh in range(2):
                    h = 2 * j + hh
                    po = hh * HD
                    qsl = qkvT_j[j]
                    ksl = qkvT_j[6 + j]
                    # K=128 zero-padded k tile: rows [po:po+64] hold this
                    # head's k, the other 64 rows are zero so the full-height
                    # q rhs contributes nothing outside this head (and the
                    # weight load takes the fast NumWeights==128 path).
                    kp = kp_pool.tile([P, NT], BF16, tag="kp")
                    nc.vector.memset(kp[HD - po:P - po, :], 0.0)
                    nc.vector.tensor_copy(kp[po:po + HD, :],
                                          ksl[po:po + HD, :])
                    ot = psot.tile([P, NT], F32, tag="ot")
                    exs = []
                    for kc in range(8):
                        st = psmm.tile([P, NT], F32, tag="mm")
                        for hs in _halves():
                            nc.tensor.matmul(
                                st[:, hs],
                                kp[:, kc * P:(kc + 1) * P],
                                qsl[:, hs],
                                start=True, stop=True)
                        ex = e_pool.tile([P, NT], BF16, tag="expst")
                        nc.scalar.activation(ex[:], st[:], AF.Exp, scale=SCALE)
                        exs.append(ex)
                        if kc >= 1:
                            exp_prev = exs[kc - 1]
                            for hs in _halves():
                                nc.tensor.matmul(
                                    ot[:, hs],
                                    vts[hh][:, (kc - 1) * P:(kc - 1) * P + P],
                                    exp_prev[:, hs],
                                    start=(kc == 1), stop=False)
                        if kc == 3 and pending_tails:
                            pending_tails.pop(0)()
                    for hs in _halves():
                        nc.tensor.matmul(ot[:, hs],
                                         vts[hh][:, 7 * P:7 * P + P],
                                         exs[7][:, hs],
                                         start=False, stop=True)
                    pending_tails.append(make_tail(h, ot))
            while pending_tails:
                pending_tails.pop(0)()
            if DEBUG:
                for k in range(KC):
                    nc.sync.dma_start(dbg["yt"][k * P:(k + 1) * P, :], yt[k][:])
        nx_stage.close()
        qkv_stage.close()

        # ---- proj + residual (per token-half so LN2 starts early) ----
        x1 = {}
        with tc.tile_pool(name="wp_pool", bufs=KC) as wp_pool:
            wp = []
            for k in range(KC):
                t = wp_pool.tile([P, D], BF16, tag="wp")
                nc.sync.dma_start(t[:], wprojT[k * P:(k + 1) * P, :])
                wp.append(t)
            for hi, hs in enumerate(_halves()):
                for m in range(KC):
                    ps = psmm.tile([P, NT], F32, tag="mm")
                    for k in range(KC):
                        nc.tensor.matmul(ps[:, hs],
                                         wp[k][:, m * P:(m + 1) * P],
                                         yt[k][:, hs],
                                         start=(k == 0), stop=(k == KC - 1))
                    xk = x1pool.tile([P, 512], F32, tag="x1")
                    # x1 = (proj_psum + proj_b) + x
                    nc.vector.scalar_tensor_tensor(
                        xk[:], ps[:, hs], pb[:, m:m + 1],
                        x_tiles[m][:, hs], ALU.add, ALU.add)
                    x1[(m, hi)] = xk
                    if DEBUG:
                        nc.sync.dma_start(dbg["x1"][m * P:(m + 1) * P, hs],
                                          xk[:])
        y_stage.close()
        stage1.close()

        # ---- LN2 ----
        nx2_stage = ctx.enter_context(ExitStack())
        nx2_pool = nx2_stage.enter_context(tc.tile_pool(name="nx2_pool",
                                                        bufs=2 * KC))
        normx2 = _layernorm(tc, lambda k, hi: x1[(k, hi)][:],
                            g2, b2, ind_sum_tiles, ind_bT,
                            invlen, eps_t, psmm, nx2_pool, "ln2")
        if DEBUG:
            for k in range(KC):
                for hi, hs in enumerate(_halves()):
                    nc.sync.dma_start(dbg["normx2"][k * P:(k + 1) * P, hs],
                                      normx2[(k, hi)][:])

        # ---- MLP ----
        h_stage = ctx.enter_context(ExitStack())
        h_pool = h_stage.enter_context(
            tc.tile_pool(name="h_pool", bufs=MFC1, side="right"))
        hT = []
        with tc.tile_pool(name="wf1_pool", bufs=KC) as wf1_pool:
            wf1 = []
            for k in range(KC):
                t = wf1_pool.tile([P, HID], BF16, tag="wfc1")
                nc.sync.dma_start(t[:], wfc1T[k * P:(k + 1) * P, :])
                wf1.append(t)
            for m in range(MFC1):
                ps = psmm.tile([P, NT], F32, tag="mm")
                for hi, hs in enumerate(_halves()):
                    for k in range(KC):
                        nc.tensor.matmul(ps[:, hs],
                                         wf1[k][:, m * P:(m + 1) * P],
                                         normx2[(k, hi)][:],
                                         start=(k == 0), stop=(k == KC - 1))
                ht = h_pool.tile([P, NT], BF16, tag="h")
                nc.scalar.activation(ht[:], ps[:], AF.Gelu,
                                     bias=f1b[:, m:m + 1])
                hT.append(ht)
                if DEBUG:
                    nc.sync.dma_start(dbg["hT"][m * P:(m + 1) * P, :], ht[:])
        nx2_stage.close()

        with tc.tile_pool(name="wf2_pool", bufs=MFC1) as wf2_pool, \
             tc.tile_pool(name="o_pool", bufs=4) as o_pool:
            wf2 = []
            for k in range(MFC1):
                t = wf2_pool.tile([P, D], BF16, tag="wfc2")
                nc.sync.dma_start(t[:], wfc2T[k * P:(k + 1) * P, :])
                wf2.append(t)
            for m in range(KC):
                ps = psmm.tile([P, NT], F32, tag="mm")
                for hs in _halves():
                    for k in range(MFC1):
                        nc.tensor.matmul(ps[:, hs],
                                         wf2[k][:, m * P:(m + 1) * P],
                                         hT[k][:, hs],
                                         start=(k == 0), stop=(k == MFC1 - 1))
                for hi, hs in enumerate(_halves()):
                    ok = o_pool.tile([P, 512], F32, tag="o")
                    # out = (fc2_psum + fc2_b) + x1
                    nc.vector.scalar_tensor_tensor(
                        ok[:], ps[:, hs], f2b[:, m:m + 1],
                        x1[(m, hi)][:], ALU.add, ALU.add)
                    nc.sync.dma_start(outT[m * P:(m + 1) * P, hs], ok[:])
        h_stage.close()

    nc.compile()
    return nc


_NC = None


def _get_nc():
    global _NC
    if _NC is None:
        _NC = build()
    return _NC


def _prep_inputs(inputs):
    f32 = np.float32
    bf16 = ml_dtypes.bfloat16
    g = {k: np.asarray(v) for k, v in inputs.items()}
    shared = {
        "wqkvT": np.ascontiguousarray(g["qkv_w"].astype(f32).T).astype(bf16),
        "wprojT": np.ascontiguousarray(g["proj_w"].astype(f32).T).astype(bf16),
        "wfc1T": np.ascontiguousarray(g["fc1_w"].astype(f32).T).astype(bf16),
        "wfc2T": np.ascontiguousarray(g["fc2_w"].astype(f32).T).astype(bf16),
        "pbias": np.ascontiguousarray(g["proj_b"], dtype=f32),
        "fc1b": np.ascontiguousarray(g["fc1_b"], dtype=f32),
        "fc2b": np.ascontiguousarray(g["fc2_b"], dtype=f32),
        "g1": np.concatenate([g["ln1a_g"], g["ln1b_g"], g["ln1c_g"]]).astype(f32),
        "b1": np.concatenate([g["ln1a_b"], g["ln1b_b"], g["ln1c_b"]]).astype(f32),
        "g2": np.concatenate([g["ln2a_g"], g["ln2b_g"], g["ln2c_g"]]).astype(f32),
        "b2": np.concatenate([g["ln2a_b"], g["ln2b_b"], g["ln2c_b"]]).astype(f32),
    }
    ind = np.zeros((D, 3), dtype=f32)
    ind[0:S1, 0] = 1.0
    ind[S1:S2, 1] = 1.0
    ind[S2:D, 2] = 1.0
    ind_pad = np.zeros((D, P), dtype=f32)
    ind_pad[:, 0:3] = ind
    shared["indsum"] = ind_pad.astype(bf16)
    shared["indbT"] = np.ascontiguousarray(ind.T)
    shared["invlen"] = np.array([[1.0 / S1], [1.0 / (S2 - S1)],
                                 [1.0 / (D - S2)]], dtype=f32)
    x = np.asarray(g["x"], dtype=f32)
    in_maps = []
    for b in range(B):
        m = dict(shared)
        m["xT"] = np.ascontiguousarray(x[b].T)
        in_maps.append(m)
    return in_maps


def run(inputs, trace=False):
    nc = _get_nc()
    in_maps = _prep_inputs(inputs)
    res = run_bass_kernel_spmd(nc, in_maps, core_ids=list(range(B)),
                               trace=trace)
    out = np.stack([np.ascontiguousarray(res.results[b]["outT"].T)
                    for b in range(B)]).astype(np.float32)
    return out, res


def kernel(**inputs):
    out, _ = run(inputs, trace=False)
    return out



# revision 25
# speedup vs baseline: 1.4514x; 1.4514x over previous
"""Trainium2 Bass kernel for a dense transformer block.

Block: split-LayerNorm -> attention -> residual -> split-LayerNorm -> MLP(GELU)
-> residual.  Shapes: B=8, N=1024, D=768, H=12 heads (hd=64), HID=3072.

Sharding: pure data-parallel over batch -- one batch element per NeuronCore
(8 cores), all weights replicated, no collectives.

On-chip layout is feature-major (activations stored transposed, [feature,
token]).  The heavy GEMMs (qkv, attn*V, proj, fc1, fc2) run in fp8e4m3 with
MatmulPerfMode.DoubleRow: weights and moving operands are packed as
[128, 2, *] slab pairs so each matmul contracts K=256 at 2x bf16 throughput.
V-transposed is produced directly on the PE (activations stationary, weights
moving), avoiding DMA transposes.  Attention scores stay bf16 with K=64
partition-sliced operands.  LayerNorm statistics use fp32r matmuls against
indicator vectors so no activation copies are needed.
"""

import os
import numpy as np
import ml_dtypes

import concourse.bass as bass
import concourse.tile as tile
from concourse import bacc, mybir
from concourse.bass_utils import run_bass_kernel_spmd
from contextlib import ExitStack

F32 = mybir.dt.float32
BF16 = mybir.dt.bfloat16
F32R = mybir.dt.float32r
U32 = mybir.dt.uint32
F8 = mybir.dt.float8e4
AF = mybir.ActivationFunctionType
ALU = mybir.AluOpType
DR = mybir.MatmulPerfMode.DoubleRow

D = 768
H = 12
HD = 64
HID = 3072
NT = 1024  # tokens per core
B = 8
S1 = 320  # split-LN segment boundaries: [0,320), [320,384), [384,768)
S2 = 384
SCALE = 0.125  # (D//H) ** -0.5 = 64 ** -0.5
EPS = 1e-5
P = 128

DEBUG = bool(int(os.environ.get("KBG_DEBUG", "0")))
DBG_SINK = None

KC = D // P        # 6  c-chunks
KP = KC // 2       # 3  c-chunk pairs
MFC1 = HID // P    # 24
MP2 = MFC1 // 2    # 12 fc2 contraction pairs
VG = 80            # per-head col group in vT tiles: 64 dims + 1 ones + pad (16B-aligned)


def _halves():
    return (slice(0, 512), slice(512, 1024))


# Per segment: (ind_bT column range for the broadcast lhsT,
#               [(chunk, row0, row1), ...] applied regions)
LN_REGIONS = [
    (slice(0, P), [(0, 0, P), (1, 0, P), (2, 0, HD)]),        # seg0 [0,320)
    (slice(S1, S2), [(2, HD, P)]),                            # seg1 [320,384)
    (slice(S2, S2 + P), [(3, 0, P), (4, 0, P), (5, 0, P)]),   # seg2 [384,768)
]


def _layernorm_half(tc, hi, xat, g_tile, b_tile, ind_sum_tiles, ind_bT,
                    invlen, eps_t, out_slab, tag):
    """Split-LayerNorm over the feature dim (partitions) for one token half.
    xat(k) returns the [128, 512] fp32 feature-major input slab for chunk k.
    out_slab(k) returns the fp8 output AP for chunk k (a [128, 512] region)."""
    nc = tc.nc
    with tc.tile_pool(name=f"ln_{tag}{hi}", bufs=3) as lnp, \
         tc.tile_pool(name=f"lnt_{tag}{hi}", bufs=KC) as lnt, \
         tc.tile_pool(name=f"lns_{tag}{hi}", bufs=1) as lns, \
         tc.tile_pool(name=f"lnps_{tag}{hi}", bufs=2, space="PSUM") as psstat, \
         tc.tile_pool(name=f"lnpb_{tag}{hi}", bufs=2, space="PSUM") as psb:
        LNMODE = os.environ.get("KBG_LN_MODE", "full")
        # segment sums via indicator matmuls: sums[s, q] = sum_{c in seg_s} x[c, q]
        sums_t = psstat.tile([P, 512], F32, tag="stat")
        sumsq_t = psstat.tile([P, 512], F32, tag="stat")
        sums = sums_t[:, :]
        sumsq = sumsq_t[:, :]
        if LNMODE != "conststats":
            for k in range(KC):
                xk = xat(k)
                xqk = lnp.tile([P, 512], F32R, tag="xq", name=f"xq_{k}")
                nc.scalar.activation(xqk[:], xk.bitcast(F32), AF.Square)
                nc.tensor.matmul(sums, ind_sum_tiles[k][:], xk,
                                 start=(k == 0), stop=(k == KC - 1))
                nc.tensor.matmul(sumsq, ind_sum_tiles[k][:], xqk[:],
                                 start=(k == 0), stop=(k == KC - 1))
        mean = lns.tile([3, 512], F32, tag="mean")
        nmsq = lns.tile([3, 512], F32, tag="nmsq")
        var = lns.tile([3, 512], F32, tag="var")
        std = lns.tile([3, 512], F32, tag="std")
        rstd = lns.tile([3, 512], F32, tag="rstd")
        beta = lns.tile([3, 512], F32, tag="beta")
        rstd_r = lns.tile([3, 512], F32R, tag="rstd_r")
        beta_r = lns.tile([3, 512], F32R, tag="beta_r")
        if LNMODE == "conststats":
            nc.vector.memset(rstd[:], 1.0)
            nc.vector.memset(beta[:], 0.0)
        else:
            nc.vector.tensor_scalar_mul(mean[:], sums_t[0:3, :], invlen[:])
            nc.vector.scalar_tensor_tensor(nmsq[:], mean[:], -1.0,
                                           mean[:], ALU.mult, ALU.mult)
            nc.vector.scalar_tensor_tensor(var[:], sumsq_t[0:3, :], invlen[:],
                                           nmsq[:], ALU.mult, ALU.add)
            nc.scalar.activation(std[:], var[:], AF.Sqrt, bias=eps_t[:])
            if LNMODE == "accrecip":
                scr = lns.tile([3, 512], F32, tag="scr")
                nc.vector.reciprocal_approx_accurate(rstd[:], std[:], scr[:])
            else:
                nc.vector.reciprocal_approx_fast(rstd[:], std[:])
            nc.vector.scalar_tensor_tensor(beta[:], mean[:], -1.0,
                                           rstd[:], ALU.mult, ALU.mult)
        nc.vector.tensor_copy(rstd_r[:], rstd[:])
        nc.vector.tensor_copy(beta_r[:], beta[:])
        if DBG_SINK is not None:
            nc.sync.dma_start(DBG_SINK["stat"][0:3, hi * 512:(hi + 1) * 512], rstd[:])
            nc.sync.dma_start(DBG_SINK["stat"][3:6, hi * 512:(hi + 1) * 512], beta[:])
        t1s = {}
        t2s = {}
        xcps = {}
        for cols, regions in LN_REGIONS:
            m_rows = max(r1 - r0 for _, r0, r1 in regions)
            aB = psb.tile([P, 512], F32, tag="mmb")
            bB = psb.tile([P, 512], F32, tag="mmb")
            if LNMODE == "conststats" and os.environ.get("KBG_LN_NOPE") == "1":
                nc.vector.memset(aB[0:m_rows, :], 1.0)
                nc.vector.memset(bB[0:m_rows, :], 0.0)
            else:
                nc.tensor.matmul(aB[0:m_rows, :], ind_bT[:, cols][:, 0:m_rows],
                                 rstd_r[:], start=True, stop=True)
                nc.tensor.matmul(bB[0:m_rows, :], ind_bT[:, cols][:, 0:m_rows],
                                 beta_r[:], start=True, stop=True)
            if DBG_SINK is not None and cols == LN_REGIONS[0][0]:
                ab_s = lnt.tile([P, 512], F32, tag="abdump", name=f"abd{hi}")
                nc.vector.tensor_copy(ab_s[:], aB[0:P, :])
                nc.sync.dma_start(DBG_SINK["ab"][:, hi * 512:(hi + 1) * 512], ab_s[:])
            if os.environ.get("KBG_LN_DIRECT") == "1":
                for k, r0, r1 in regions:
                    if r1 == P:
                        nc.scalar.activation(out_slab(k),
                                             xat(k).bitcast(F32), AF.Identity,
                                             bias=b_tile[:, k:k + 1])
                continue
            for k, r0, r1 in regions:
                if k not in t1s:
                    t1s[k] = lnt.tile([P, 512], F32, tag="t1", name=f"t1_{k}")
                    t2s[k] = lnt.tile([P, 512], F32, tag="t2", name=f"t2_{k}")
                gcol = g_tile[r0:r1, k:k + 1]
                if os.environ.get("KBG_LN_XU32", "0") == "1":
                    if k not in xcps:
                        xcps[k] = lnp.tile([P, 512], F32, tag="xcp",
                                           name=f"xcp_{k}_{hi}")
                        nc.vector.tensor_copy(xcps[k][:].bitcast(U32),
                                              xat(k).bitcast(U32))
                    xin = xcps[k][r0:r1, :]
                else:
                    xin = xat(k)[r0:r1, :].bitcast(F32)
                # t1 = (x * g) * rstd_bcast ; t2 = (-mean*rstd*g) + t1
                nc.vector.scalar_tensor_tensor(
                    t1s[k][r0:r1, :], xin, gcol,
                    aB[0:r1 - r0, :], ALU.mult, ALU.mult)
                nc.vector.scalar_tensor_tensor(
                    t2s[k][r0:r1, :], bB[0:r1 - r0, :], gcol,
                    t1s[k][r0:r1, :], ALU.mult, ALU.add)
                if r1 == P:
                    if DBG_SINK is not None and k == 0 and "t2" in DBG_SINK:
                        nc.sync.dma_start(
                            DBG_SINK["t2"][:, hi * 512:(hi + 1) * 512],
                            t2s[k][:])
                    if os.environ.get("KBG_LN_DVECOPY", "0") == "1":
                        nc.vector.tensor_copy(out_slab(k), t2s[k][:])
                    else:
                        nc.scalar.activation(out_slab(k), t2s[k][:], AF.Identity,
                                             bias=b_tile[:, k:k + 1])


def build():
    nc = bacc.Bacc("TRN2", target_bir_lowering=False, debug=False)

    xT = nc.dram_tensor("xT", [D, NT], F32R, kind="ExternalInput")
    wqk8d = nc.dram_tensor("wqk8", [KP * P, 2 * 2 * D], F8, kind="ExternalInput")
    wv8d = nc.dram_tensor("wv8", [KP * P, 2 * D], F8, kind="ExternalInput")
    wp8d = nc.dram_tensor("wp8", [KP * P, 2 * D], F8, kind="ExternalInput")
    wf18d = nc.dram_tensor("wf18", [KP * P, 2 * HID], F8, kind="ExternalInput")
    wf28d = nc.dram_tensor("wf28", [MP2 * P, 2 * D], F8, kind="ExternalInput")
    pbias = nc.dram_tensor("pbias", [D], F32, kind="ExternalInput")
    fc1b = nc.dram_tensor("fc1b", [HID], F32, kind="ExternalInput")
    fc2b = nc.dram_tensor("fc2b", [D], F32, kind="ExternalInput")
    g1d = nc.dram_tensor("g1", [D], F32, kind="ExternalInput")
    b1d = nc.dram_tensor("b1", [D], F32, kind="ExternalInput")
    g2d = nc.dram_tensor("g2", [D], F32, kind="ExternalInput")
    b2d = nc.dram_tensor("b2", [D], F32, kind="ExternalInput")
    indsum = nc.dram_tensor("indsum", [D, P], F32, kind="ExternalInput")
    indbTd = nc.dram_tensor("indbT", [3, D], F32, kind="ExternalInput")
    invlend = nc.dram_tensor("invlen", [3, 1], F32, kind="ExternalInput")
    outT = nc.dram_tensor("outT", [D, NT], F32, kind="ExternalOutput")
    dbg = {}
    if DEBUG:
        dbg["nxp"] = nc.dram_tensor("dbg_nxp", [KP * P, 2 * NT], F8, kind="ExternalOutput")
        dbg["qt"] = nc.dram_tensor("dbg_qt", [12 * P, NT], BF16, kind="ExternalOutput")
        dbg["vtp"] = nc.dram_tensor("dbg_vtp", [4 * P, 2 * H * VG], F8, kind="ExternalOutput")
        dbg["ex0"] = nc.dram_tensor("dbg_ex0", [4 * P, 2 * NT], F8, kind="ExternalOutput")
        dbg["ytp"] = nc.dram_tensor("dbg_ytp", [KP * P, 2 * NT], F8, kind="ExternalOutput")
        dbg["x1"] = nc.dram_tensor("dbg_x1", [D, NT], F32R, kind="ExternalOutput")
        dbg["recip"] = nc.dram_tensor("dbg_recip", [H, NT], F32, kind="ExternalOutput")
        dbg["dnm"] = nc.dram_tensor("dbg_dnm", [H, NT], F32, kind="ExternalOutput")

    with tile.TileContext(nc) as tc, ExitStack() as ctx:
        const = ctx.enter_context(tc.tile_pool(name="const", bufs=1))

        # constants
        eps_t = const.tile([3, 1], F32)
        nc.vector.memset(eps_t[:], EPS)
        # prewarm activation tables while the x DMA streams in
        warm = const.tile([1, 1], F32)
        nc.vector.memset(warm[:], 0.25)
        for fn in (AF.Square, AF.Sqrt, AF.Exp, AF.Gelu, AF.Identity):
            wo = const.tile([1, 1], F32, tag=f"warm_{fn}")
            nc.scalar.activation(wo[:], warm[:], fn)

        # x first: LN1 start gates everything
        x0pool = ctx.enter_context(tc.tile_pool(name="x0pool", bufs=KC))
        x_tiles = []
        for k in range(KC):
            t = x0pool.tile([P, NT], F32R, tag="x0")
            for qr in range(4):
                nc.sync.dma_start(t[qr * 32:(qr + 1) * 32, :],
                                  xT[k * P + qr * 32:k * P + (qr + 1) * 32, :])
            x_tiles.append(t)

        def load_cols(dram, n):
            t = const.tile([P, n], F32, tag=f"c_{dram.name}")
            nc.sync.dma_start(t[:], dram.ap().rearrange("(a p) -> p a", p=P))
            return t

        pb = load_cols(pbias, KC)
        f1b = load_cols(fc1b, MFC1)
        f2b = load_cols(fc2b, KC)
        g1 = load_cols(g1d, KC)
        b1 = load_cols(b1d, KC)
        g2 = load_cols(g2d, KC)
        b2 = load_cols(b2d, KC)
        ind_sum_tiles = []
        for k in range(KC):
            tf = const.tile([P, P], F32, tag=f"indsf{k}")
            nc.sync.dma_start(tf[:], indsum[k * P:(k + 1) * P, :])
            t = const.tile([P, P], F32R, tag=f"inds{k}")
            nc.vector.tensor_copy(t[:], tf[:])
            ind_sum_tiles.append(t)
        ind_bTf = const.tile([3, D], F32)
        nc.sync.dma_start(ind_bTf[:], indbTd[:])
        ind_bT = const.tile([3, D], F32R)
        nc.vector.tensor_copy(ind_bT[:], ind_bTf[:])
        invlen = const.tile([3, 1], F32)
        nc.sync.dma_start(invlen[:], invlend[:])

        # proj weights first (pool outlives the attention-weight pool)
        wlate = ctx.enter_context(ExitStack())
        wp_pool = wlate.enter_context(tc.tile_pool(name="wp_pool", bufs=KP))
        wp8 = []
        for k2 in range(KP):
            t = wp_pool.tile([P, 2, D], F8, tag="wp8")
            nc.sync.dma_start(t[:], wp8d[k2 * P:(k2 + 1) * P, :])
            wp8.append(t)
        # attention weights (needed soon after LN1)
        wq_stage = ctx.enter_context(ExitStack())
        wq_pool = wq_stage.enter_context(tc.tile_pool(name="wqk", bufs=2 * KP))
        wqk8 = []
        wv8 = []
        for k2 in range(KP):
            t = wq_pool.tile([P, 2, 2 * D], F8, tag="wqk8")
            nc.sync.dma_start(t[:], wqk8d[k2 * P:(k2 + 1) * P, :])
            wqk8.append(t)
        for k2 in range(KP):
            t = wq_pool.tile([P, 2, D], F8, tag="wv8")
            nc.sync.dma_start(t[:], wv8d[k2 * P:(k2 + 1) * P, :])
            wv8.append(t)

        # ---- LN1 ----
        v_stage = ctx.enter_context(ExitStack())   # vT tiles, live thru attention
        v_pool = v_stage.enter_context(tc.tile_pool(name="v_pool", bufs=4))
        nx_stage = ctx.enter_context(ExitStack())  # normx pairs, live thru qkv/vT
        nx_pool = nx_stage.enter_context(tc.tile_pool(name="nx_pool", bufs=KP))
        nxp = [nx_pool.tile([P, 2, NT], F8, tag="nxp", name=f"nxp{i}")
           for i in range(KP)]
        for hi in range(2):
            hs = _halves()[hi]
            _layernorm_half(tc, hi,
                            lambda k, hs=hs: x_tiles[k][:, hs],
                            g1, b1, ind_sum_tiles, ind_bT,
                            invlen, eps_t,
                            lambda k, hs=hs: nxp[k // 2][:, k % 2, hs],
                            "ln1")

        if DEBUG:
            for k2 in range(KP):
                nc.sync.dma_start(dbg["nxp"][k2 * P:(k2 + 1) * P, :], nxp[k2][:])

        # ---- qkv (q,k) in fp8 DoubleRow; vT via PE (weights moving) ----
        y_stage = ctx.enter_context(ExitStack())
        y_pool = y_stage.enter_context(
            tc.tile_pool(name="y_pool", bufs=KP, side="right"))
        ytp = [y_pool.tile([P, 2, NT], F8, tag="ytp", name=f"ytp{i}")
           for i in range(KP)]
        qkv_stage = ctx.enter_context(ExitStack())
        q_pool = qkv_stage.enter_context(
            tc.tile_pool(name="q_pool", bufs=12, side="right"))

        psmm_stage = ctx.enter_context(ExitStack())
        psmm = psmm_stage.enter_context(
            tc.tile_pool(name="psmm", bufs=2, space="PSUM"))

        qt = []
        for m in range(2 * KC):  # q chunks 0..5, k chunks 6..11
            ps = psmm.tile([P, NT], F32, tag="mm")
            for hi, hs in enumerate(_halves()):
                for k2 in range(KP):
                    nc.tensor.matmul(ps[:, hs],
                                     wqk8[k2][:, :, m * P:(m + 1) * P],
                                     nxp[k2][:, :, hs],
                                     start=(k2 == 0), stop=(k2 == KP - 1),
                                     perf_mode=DR)
            t = q_pool.tile([P, NT], BF16, tag="qkt")
            nc.vector.tensor_copy(t[:], ps[:])
            qt.append(t)
            if DEBUG:
                nc.sync.dma_start(dbg["qt"][m * P:(m + 1) * P, :], t[:])

        # vT: out[tok, vdim] = sum_feat normx[feat, tok] * wv[feat, vdim]
        vtp = []
        for p2 in range(4):
            vt = v_pool.tile([P, 2, H * VG], F8, tag="vtp")
            # ones columns (col 64 of each 66-wide head group) for denominators
            for s in range(2):
                nc.vector.memset(
                    vt[:, s, :].rearrange("p (h c) -> p h c", c=VG)[:, :, HD:VG],
                    1.0)
            vtp.append(vt)
        with tc.tile_pool(name="vtps", bufs=2, space="PSUM") as vtps:
            for tc_i in range(8):
                vps = vtps.tile([P, D], F32, tag="vps")
                for cs in (slice(0, 512), slice(512, D)):
                    for k2 in range(KP):
                        nc.tensor.matmul(vps[:, cs],
                                         nxp[k2][:, :, tc_i * P:(tc_i + 1) * P],
                                         wv8[k2][:, :, cs],
                                         start=(k2 == 0), stop=(k2 == KP - 1),
                                         perf_mode=DR)
                dst = vtp[tc_i // 2][:, tc_i % 2, :].rearrange(
                    "p (h c) -> p h c", c=VG)[:, :, 0:HD]
                nc.vector.tensor_copy(
                    dst, vps[:].rearrange("p (h c) -> p h c", c=HD))

        if DEBUG:
            for p2 in range(4):
                nc.sync.dma_start(dbg["vtp"][p2 * P:(p2 + 1) * P, :], vtp[p2][:])

        # ---- attention, head by head ----
        with tc.tile_pool(name="e_pool", bufs=6) as e_pool, \
             tc.tile_pool(name="psot", bufs=2, space="PSUM") as psot, \
             tc.tile_pool(name="sm_pool", bufs=2) as sm_pool:

            def make_tail(h, ot):
                def tail():
                    dnm = sm_pool.tile([1, NT], F32, tag="dnm")
                    nc.vector.tensor_copy(dnm[:], ot[HD:HD + 1, :])
                    r = sm_pool.tile([1, NT], F32, tag="recip")
                    nc.vector.reciprocal_approx_fast(r[:], dnm[:])
                    if DEBUG:
                        nc.sync.dma_start(dbg["dnm"][h:h + 1, :], dnm[:])
                        nc.sync.dma_start(dbg["recip"][h:h + 1, :], r[:])
                    rbs = sm_pool.tile([HD, NT], F32, tag="rbs")
                    nc.gpsimd.partition_broadcast(rbs[:], r[:])
                    po2 = (h % 2) * HD
                    nc.vector.tensor_mul(
                        ytp[h // 4][po2:po2 + HD, (h // 2) % 2, :],
                        ot[0:HD, :], rbs[:])
                return tail

            pending_av = []
            pending_tails = []
            for h in range(H):
                j = h // 2
                po = (h % 2) * HD
                qsl = qt[j]
                ksl = qt[KC + j]
                ot = psot.tile([P, NT], F32, tag="ot")
                exs = [e_pool.tile([P, 2, NT], F8, tag="ex", name=f"ex{h}_{i}")
                       for i in range(4)]

                def av(p2, ot=ot, exs=exs, hh=h):
                    for hs in _halves():
                        nc.tensor.matmul(
                            ot[0:HD + 2, hs],
                            vtp[p2][:, :, hh * VG:hh * VG + HD + 2],
                            exs[p2][:, :, hs],
                            start=(p2 == 0), stop=(p2 == 3),
                            perf_mode=DR)

                for kc in range(8):
                    st = psmm.tile([P, NT], F32, tag="mm")
                    for hs in _halves():
                        nc.tensor.matmul(
                            st[:, hs],
                            ksl[po:po + HD, kc * P:(kc + 1) * P],
                            qsl[po:po + HD, hs],
                            start=True, stop=True)
                    nc.scalar.activation(exs[kc // 2][:, kc % 2, :],
                                         st[:], AF.Exp, scale=SCALE)
                    if kc == 1 and pending_av:
                        pending_av.pop(0)()
                    if kc == 4 and pending_tails:
                        pending_tails.pop(0)()
                    if kc in (3, 5, 7):
                        av((kc - 3) // 2)
                pending_av.append(lambda av=av: av(3))
                pending_tails.append(make_tail(h, ot))
                if DEBUG and h == 0:
                    for p2 in range(4):
                        nc.sync.dma_start(dbg["ex0"][p2 * P:(p2 + 1) * P, :],
                                          exs[p2][:])
            while pending_av:
                pending_av.pop(0)()
            while pending_tails:
                pending_tails.pop(0)()
        if DEBUG:
            for k2 in range(KP):
                nc.sync.dma_start(dbg["ytp"][k2 * P:(k2 + 1) * P, :], ytp[k2][:])
        nx_stage.close()
        v_stage.close()
        wq_stage.close()
        qkv_stage.close()
        psmm_stage.close()

        # MLP weights: issue DMAs now (consumed ~20us later)
        wf1_pool = wlate.enter_context(tc.tile_pool(name="wf1_pool", bufs=KP))
        wf2_pool = wlate.enter_context(tc.tile_pool(name="wf2_pool", bufs=MP2))
        wf18 = []
        for k2 in range(KP):
            t = wf1_pool.tile([P, 2, HID], F8, tag="wf18")
            nc.sync.dma_start(t[:], wf18d[k2 * P:(k2 + 1) * P, :])
            wf18.append(t)
        wf28 = []
        for m2 in range(MP2):
            t = wf2_pool.tile([P, 2, D], F8, tag="wf28")
            nc.sync.dma_start(t[:], wf28d[m2 * P:(m2 + 1) * P, :])
            wf28.append(t)
        x1pool = ctx.enter_context(tc.tile_pool(name="x1pool", bufs=2 * KC))

        # ---- proj + residual (per token-half so LN2 starts early) ----
        x1 = {}
        with tc.tile_pool(name="pspj", bufs=2, space="PSUM") as pspj:
            for hi, hs in enumerate(_halves()):
                for m in range(KC):
                    ps = pspj.tile([P, 512], F32, tag="pj")
                    for k2 in range(KP):
                        nc.tensor.matmul(ps[:],
                                         wp8[k2][:, :, m * P:(m + 1) * P],
                                         ytp[k2][:, :, hs],
                                         start=(k2 == 0), stop=(k2 == KP - 1),
                                         perf_mode=DR)
                    xk = x1pool.tile([P, 512], F32R, tag="x1")
                    # x1 = (proj_psum + proj_b) + x
                    nc.vector.scalar_tensor_tensor(
                        xk[:], ps[:], pb[:, m:m + 1],
                        x_tiles[m][:, hs].bitcast(F32), ALU.add, ALU.add)
                    x1[(m, hi)] = xk
                    if DEBUG:
                        nc.sync.dma_start(dbg["x1"][m * P:(m + 1) * P, hs], xk[:])
        y_stage.close()

        # ---- LN2 + MLP, pipelined per token-half ----
        nx2_stage = ctx.enter_context(ExitStack())
        nx2_pool = nx2_stage.enter_context(tc.tile_pool(name="nx2_pool", bufs=KP))
        nx2p = [nx2_pool.tile([P, 2, NT], F8, tag="nx2p", name=f"nx2p{i}")
            for i in range(KP)]
        h_stage = ctx.enter_context(ExitStack())
        h_pool = h_stage.enter_context(
            tc.tile_pool(name="h_pool", bufs=MP2, side="right"))
        h8 = [h_pool.tile([P, 2, NT], F8, tag="h8", name=f"h8_{i}")
          for i in range(MP2)]

        with tc.tile_pool(name="psmlp", bufs=4, space="PSUM") as psmlp, \
             tc.tile_pool(name="o_pool", bufs=4) as o_pool:
            for hi in range(2):
                hs = _halves()[hi]
                _layernorm_half(tc, hi, lambda k, hi=hi: x1[(k, hi)][:],
                                g2, b2, ind_sum_tiles, ind_bT,
                                invlen, eps_t,
                                lambda k, hs=hs: nx2p[k // 2][:, k % 2, hs],
                                "ln2")
                # fc1 + gelu for this half
                for m in range(MFC1):
                    ps = psmlp.tile([P, 512], F32, tag="mlp")
                    for k2 in range(KP):
                        nc.tensor.matmul(ps[:],
                                         wf18[k2][:, :, m * P:(m + 1) * P],
                                         nx2p[k2][:, :, hs],
                                         start=(k2 == 0), stop=(k2 == KP - 1),
                                         perf_mode=DR)
                    nc.scalar.activation(h8[m // 2][:, m % 2, hs], ps[:],
                                         AF.Gelu, bias=f1b[:, m:m + 1])
                # fc2 + residual for this half
                for m in range(KC):
                    ps = psmlp.tile([P, 512], F32, tag="mlp")
                    for m2 in range(MP2):
                        nc.tensor.matmul(ps[:],
                                         wf28[m2][:, :, m * P:(m + 1) * P],
                                         h8[m2][:, :, hs],
                                         start=(m2 == 0), stop=(m2 == MP2 - 1),
                                         perf_mode=DR)
                    ok = o_pool.tile([P, 512], F32, tag="o")
                    # out = (fc2_psum + fc2_b) + x1
                    nc.vector.scalar_tensor_tensor(
                        ok[:], ps[:], f2b[:, m:m + 1],
                        x1[(m, hi)][:].bitcast(F32), ALU.add, ALU.add)
                    nc.sync.dma_start(outT[m * P:(m + 1) * P, hs], ok[:])

    nc.compile()
    return nc


_NC = None


def _get_nc():
    global _NC
    if _NC is None:
        _NC = build()
    return _NC


def _pair_rows(wT):
    """[K, F] fp32 row-major -> [K//2, 2*F] fp8 with 128-row slab pairs:
    out[p*128+i, s*F+m] = wT[256*p + 128*s + i, m]."""
    K, F = wT.shape
    return np.ascontiguousarray(
        wT.reshape(K // 256, 2, 128, F).transpose(0, 2, 1, 3).reshape(K // 2, 2 * F)
    ).astype(ml_dtypes.float8_e4m3)


def _prep_inputs(inputs):
    f32 = np.float32
    g = {k: np.asarray(v) for k, v in inputs.items()}
    qkvT = np.ascontiguousarray(g["qkv_w"].astype(f32).T)  # [D, 3D]
    shared = {
        "wqk8": _pair_rows(qkvT[:, 0:2 * D]),
        "wv8": _pair_rows(qkvT[:, 2 * D:3 * D]),
        "wp8": _pair_rows(np.ascontiguousarray(g["proj_w"].astype(f32).T)),
        "wf18": _pair_rows(np.ascontiguousarray(g["fc1_w"].astype(f32).T)),
        "wf28": _pair_rows(np.ascontiguousarray(g["fc2_w"].astype(f32).T)),
        "pbias": np.ascontiguousarray(g["proj_b"], dtype=f32),
        "fc1b": np.ascontiguousarray(g["fc1_b"], dtype=f32),
        "fc2b": np.ascontiguousarray(g["fc2_b"], dtype=f32),
        "g1": np.concatenate([g["ln1a_g"], g["ln1b_g"], g["ln1c_g"]]).astype(f32),
        "b1": np.concatenate([g["ln1a_b"], g["ln1b_b"], g["ln1c_b"]]).astype(f32),
        "g2": np.concatenate([g["ln2a_g"], g["ln2b_g"], g["ln2c_g"]]).astype(f32),
        "b2": np.concatenate([g["ln2a_b"], g["ln2b_b"], g["ln2c_b"]]).astype(f32),
    }
    ind = np.zeros((D, 128), dtype=f32)
    ind[0:S1, 0] = 1.0
    ind[S1:S2, 1] = 1.0
    ind[S2:D, 2] = 1.0
    shared["indsum"] = ind
    shared["indbT"] = np.ascontiguousarray(ind[:, 0:3].T)
    shared["invlen"] = np.array([[1.0 / S1], [1.0 / (S2 - S1)],
                                 [1.0 / (D - S2)]], dtype=f32)
    x = np.asarray(g["x"], dtype=f32)
    in_maps = []
    for b in range(B):
        m = dict(shared)
        m["xT"] = np.ascontiguousarray(x[b].T)
        in_maps.append(m)
    return in_maps


def run(inputs, trace=False):
    nc = _get_nc()
    in_maps = _prep_inputs(inputs)
    res = run_bass_kernel_spmd(nc, in_maps, core_ids=list(range(B)),
                               trace=trace)
    out = np.stack([np.ascontiguousarray(res.results[b]["outT"].T)
                    for b in range(B)]).astype(np.float32)
    return out, res


def kernel(**inputs):
    out, _ = run(inputs, trace=False)
    return out


# revision 28
# speedup vs baseline: 1.4574x; 1.0042x over previous
"""Trainium2 Bass kernel for a dense transformer block.

Block: split-LayerNorm -> attention -> residual -> split-LayerNorm -> MLP(GELU)
-> residual.  Shapes: B=8, N=1024, D=768, H=12 heads (hd=64), HID=3072.

Sharding: pure data-parallel over batch -- one batch element per NeuronCore
(8 cores), all weights replicated, no collectives.

On-chip layout is feature-major (activations stored transposed, [feature,
token]).  The heavy GEMMs (qkv, attn*V, proj, fc1, fc2) run in fp8e4m3 with
MatmulPerfMode.DoubleRow: weights and moving operands are packed as
[128, 2, *] slab pairs so each matmul contracts K=256 at 2x bf16 throughput.
V-transposed is produced directly on the PE (activations stationary, weights
moving), avoiding DMA transposes.  Attention scores stay bf16 with K=64
partition-sliced operands.  LayerNorm statistics use fp32r matmuls against
indicator vectors so no activation copies are needed.
"""

import os
import numpy as np
import ml_dtypes

import concourse.bass as bass
import concourse.tile as tile
from concourse import bacc, mybir
from concourse.bass_utils import run_bass_kernel_spmd
from contextlib import ExitStack

F32 = mybir.dt.float32
BF16 = mybir.dt.bfloat16
F32R = mybir.dt.float32r
U32 = mybir.dt.uint32
F8 = mybir.dt.float8e4
AF = mybir.ActivationFunctionType
ALU = mybir.AluOpType
DR = mybir.MatmulPerfMode.DoubleRow

D = 768
H = 12
HD = 64
HID = 3072
NT = 1024  # tokens per core
B = 8
S1 = 320  # split-LN segment boundaries: [0,320), [320,384), [384,768)
S2 = 384
SCALE = 0.125  # (D//H) ** -0.5 = 64 ** -0.5
EPS = 1e-5
P = 128

DEBUG = bool(int(os.environ.get("KBG_DEBUG", "0")))
DBG_SINK = None

KC = D // P        # 6  c-chunks
KP = KC // 2       # 3  c-chunk pairs
MFC1 = HID // P    # 24
MP2 = MFC1 // 2    # 12 fc2 contraction pairs
VG = 80            # per-head col group in vT tiles: 64 dims + 1 ones + pad (16B-aligned)


def _halves():
    return (slice(0, 512), slice(512, 1024))


# Per segment: (ind_bT column range for the broadcast lhsT,
#               [(chunk, row0, row1), ...] applied regions)
LN_REGIONS = [
    (slice(0, P), [(0, 0, P), (1, 0, P), (2, 0, HD)]),        # seg0 [0,320)
    (slice(S1, S2), [(2, HD, P)]),                            # seg1 [320,384)
    (slice(S2, S2 + P), [(3, 0, P), (4, 0, P), (5, 0, P)]),   # seg2 [384,768)
]


def _layernorm_half(tc, hi, xat, g_tile, b_tile, ind_sum_tiles, ind_bT,
                    invlen, eps_t, out_slab, tag):
    """Split-LayerNorm over the feature dim (partitions) for one token half.
    xat(k) returns the [128, 512] fp32 feature-major input slab for chunk k.
    out_slab(k) returns the fp8 output AP for chunk k (a [128, 512] region)."""
    nc = tc.nc
    with tc.tile_pool(name=f"ln_{tag}{hi}", bufs=3) as lnp, \
         tc.tile_pool(name=f"lnt_{tag}{hi}", bufs=KC) as lnt, \
         tc.tile_pool(name=f"lns_{tag}{hi}", bufs=1) as lns, \
         tc.tile_pool(name=f"lnps_{tag}{hi}", bufs=2, space="PSUM") as psstat, \
         tc.tile_pool(name=f"lnpb_{tag}{hi}", bufs=2, space="PSUM") as psb:
        LNMODE = os.environ.get("KBG_LN_MODE", "full")
        # segment sums via indicator matmuls: sums[s, q] = sum_{c in seg_s} x[c, q]
        sums_t = psstat.tile([P, 512], F32, tag="stat")
        sumsq_t = psstat.tile([P, 512], F32, tag="stat")
        sums = sums_t[:, :]
        sumsq = sumsq_t[:, :]
        if LNMODE != "conststats":
            for k in range(KC):
                xk = xat(k)
                xqk = lnp.tile([P, 512], F32R, tag="xq", name=f"xq_{k}")
                nc.scalar.activation(xqk[:], xk.bitcast(F32), AF.Square)
                nc.tensor.matmul(sums, ind_sum_tiles[k][:], xk,
                                 start=(k == 0), stop=(k == KC - 1))
                nc.tensor.matmul(sumsq, ind_sum_tiles[k][:], xqk[:],
                                 start=(k == 0), stop=(k == KC - 1))
        mean = lns.tile([3, 512], F32, tag="mean")
        nmsq = lns.tile([3, 512], F32, tag="nmsq")
        var = lns.tile([3, 512], F32, tag="var")
        std = lns.tile([3, 512], F32, tag="std")
        rstd = lns.tile([3, 512], F32, tag="rstd")
        beta = lns.tile([3, 512], F32, tag="beta")
        rstd_r = lns.tile([3, 512], F32R, tag="rstd_r")
        beta_r = lns.tile([3, 512], F32R, tag="beta_r")
        if LNMODE == "conststats":
            nc.vector.memset(rstd[:], 1.0)
            nc.vector.memset(beta[:], 0.0)
        else:
            nc.vector.tensor_scalar_mul(mean[:], sums_t[0:3, :], invlen[:])
            nc.vector.scalar_tensor_tensor(nmsq[:], mean[:], -1.0,
                                           mean[:], ALU.mult, ALU.mult)
            nc.vector.scalar_tensor_tensor(var[:], sumsq_t[0:3, :], invlen[:],
                                           nmsq[:], ALU.mult, ALU.add)
            nc.scalar.activation(std[:], var[:], AF.Sqrt, bias=eps_t[:])
            if LNMODE == "accrecip":
                scr = lns.tile([3, 512], F32, tag="scr")
                nc.vector.reciprocal_approx_accurate(rstd[:], std[:], scr[:])
            else:
                nc.vector.reciprocal_approx_fast(rstd[:], std[:])
            nc.vector.scalar_tensor_tensor(beta[:], mean[:], -1.0,
                                           rstd[:], ALU.mult, ALU.mult)
        nc.vector.tensor_copy(rstd_r[:], rstd[:])
        nc.vector.tensor_copy(beta_r[:], beta[:])
        if DBG_SINK is not None:
            nc.sync.dma_start(DBG_SINK["stat"][0:3, hi * 512:(hi + 1) * 512], rstd[:])
            nc.sync.dma_start(DBG_SINK["stat"][3:6, hi * 512:(hi + 1) * 512], beta[:])
        t1s = {}
        t2s = {}
        xcps = {}
        for cols, regions in LN_REGIONS:
            m_rows = max(r1 - r0 for _, r0, r1 in regions)
            aB = psb.tile([P, 512], F32, tag="mmb")
            bB = psb.tile([P, 512], F32, tag="mmb")
            if LNMODE == "conststats" and os.environ.get("KBG_LN_NOPE") == "1":
                nc.vector.memset(aB[0:m_rows, :], 1.0)
                nc.vector.memset(bB[0:m_rows, :], 0.0)
            else:
                nc.tensor.matmul(aB[0:m_rows, :], ind_bT[:, cols][:, 0:m_rows],
                                 rstd_r[:], start=True, stop=True)
                nc.tensor.matmul(bB[0:m_rows, :], ind_bT[:, cols][:, 0:m_rows],
                                 beta_r[:], start=True, stop=True)
            if DBG_SINK is not None and cols == LN_REGIONS[0][0]:
                ab_s = lnt.tile([P, 512], F32, tag="abdump", name=f"abd{hi}")
                nc.vector.tensor_copy(ab_s[:], aB[0:P, :])
                nc.sync.dma_start(DBG_SINK["ab"][:, hi * 512:(hi + 1) * 512], ab_s[:])
            if os.environ.get("KBG_LN_DIRECT") == "1":
                for k, r0, r1 in regions:
                    if r1 == P:
                        nc.scalar.activation(out_slab(k),
                                             xat(k).bitcast(F32), AF.Identity,
                                             bias=b_tile[:, k:k + 1])
                continue
            for k, r0, r1 in regions:
                if k not in t1s:
                    t1s[k] = lnt.tile([P, 512], F32, tag="t1", name=f"t1_{k}")
                    t2s[k] = lnt.tile([P, 512], F32, tag="t2", name=f"t2_{k}")
                gcol = g_tile[r0:r1, k:k + 1]
                if os.environ.get("KBG_LN_XU32", "0") == "1":
                    if k not in xcps:
                        xcps[k] = lnp.tile([P, 512], F32, tag="xcp",
                                           name=f"xcp_{k}_{hi}")
                        nc.vector.tensor_copy(xcps[k][:].bitcast(U32),
                                              xat(k).bitcast(U32))
                    xin = xcps[k][r0:r1, :]
                else:
                    xin = xat(k)[r0:r1, :].bitcast(F32)
                # t1 = (x * g) * rstd_bcast ; t2 = (-mean*rstd*g) + t1
                nc.vector.scalar_tensor_tensor(
                    t1s[k][r0:r1, :], xin, gcol,
                    aB[0:r1 - r0, :], ALU.mult, ALU.mult)
                nc.vector.scalar_tensor_tensor(
                    t2s[k][r0:r1, :], bB[0:r1 - r0, :], gcol,
                    t1s[k][r0:r1, :], ALU.mult, ALU.add)
                if r1 == P:
                    if DBG_SINK is not None and k == 0 and "t2" in DBG_SINK:
                        nc.sync.dma_start(
                            DBG_SINK["t2"][:, hi * 512:(hi + 1) * 512],
                            t2s[k][:])
                    if os.environ.get("KBG_LN_DVECOPY", "0") == "1":
                        nc.vector.tensor_copy(out_slab(k), t2s[k][:])
                    else:
                        nc.scalar.activation(out_slab(k), t2s[k][:], AF.Identity,
                                             bias=b_tile[:, k:k + 1])


def build():
    nc = bacc.Bacc("TRN2", target_bir_lowering=False, debug=False)

    xT = nc.dram_tensor("xT", [D, NT], F32R, kind="ExternalInput")
    wqk8d = nc.dram_tensor("wqk8", [KP * P, 2 * 2 * D], F8, kind="ExternalInput")
    wv8d = nc.dram_tensor("wv8", [KP * P, 2 * D], F8, kind="ExternalInput")
    wp8d = nc.dram_tensor("wp8", [KP * P, 2 * D], F8, kind="ExternalInput")
    wf18d = nc.dram_tensor("wf18", [KP * P, 2 * HID], F8, kind="ExternalInput")
    wf28d = nc.dram_tensor("wf28", [MP2 * P, 2 * D], F8, kind="ExternalInput")
    pbias = nc.dram_tensor("pbias", [D], F32, kind="ExternalInput")
    fc1b = nc.dram_tensor("fc1b", [HID], F32, kind="ExternalInput")
    fc2b = nc.dram_tensor("fc2b", [D], F32, kind="ExternalInput")
    g1d = nc.dram_tensor("g1", [D], F32, kind="ExternalInput")
    b1d = nc.dram_tensor("b1", [D], F32, kind="ExternalInput")
    g2d = nc.dram_tensor("g2", [D], F32, kind="ExternalInput")
    b2d = nc.dram_tensor("b2", [D], F32, kind="ExternalInput")
    indsum = nc.dram_tensor("indsum", [D, P], F32, kind="ExternalInput")
    indbTd = nc.dram_tensor("indbT", [3, D], F32, kind="ExternalInput")
    invlend = nc.dram_tensor("invlen", [3, 1], F32, kind="ExternalInput")
    outT = nc.dram_tensor("outT", [D, NT], F32, kind="ExternalOutput")
    dbg = {}
    if DEBUG:
        dbg["nxp"] = nc.dram_tensor("dbg_nxp", [KP * P, 2 * NT], F8, kind="ExternalOutput")
        dbg["qt"] = nc.dram_tensor("dbg_qt", [12 * P, NT], BF16, kind="ExternalOutput")
        dbg["vtp"] = nc.dram_tensor("dbg_vtp", [4 * P, 2 * H * VG], F8, kind="ExternalOutput")
        dbg["ex0"] = nc.dram_tensor("dbg_ex0", [4 * P, 2 * NT], F8, kind="ExternalOutput")
        dbg["ytp"] = nc.dram_tensor("dbg_ytp", [KP * P, 2 * NT], F8, kind="ExternalOutput")
        dbg["x1"] = nc.dram_tensor("dbg_x1", [D, NT], F32R, kind="ExternalOutput")
        dbg["recip"] = nc.dram_tensor("dbg_recip", [H, NT], F32, kind="ExternalOutput")
        dbg["dnm"] = nc.dram_tensor("dbg_dnm", [H, NT], F32, kind="ExternalOutput")

    with tile.TileContext(nc) as tc, ExitStack() as ctx:
        const = ctx.enter_context(tc.tile_pool(name="const", bufs=1))

        # constants
        eps_t = const.tile([3, 1], F32)
        nc.vector.memset(eps_t[:], EPS)
        # prewarm activation tables while the x DMA streams in
        warm = const.tile([1, 1], F32)
        nc.vector.memset(warm[:], 0.25)
        for fn in (AF.Square, AF.Sqrt, AF.Exp, AF.Gelu, AF.Identity):
            wo = const.tile([1, 1], F32, tag=f"warm_{fn}")
            nc.scalar.activation(wo[:], warm[:], fn)

        # x first: LN1 start gates everything
        x0pool = ctx.enter_context(tc.tile_pool(name="x0pool", bufs=KC))
        x_tiles = []
        for k in range(KC):
            t = x0pool.tile([P, NT], F32R, tag="x0")
            for qr in range(4):
                nc.sync.dma_start(t[qr * 32:(qr + 1) * 32, :],
                                  xT[k * P + qr * 32:k * P + (qr + 1) * 32, :])
            x_tiles.append(t)

        def load_cols(dram, n):
            t = const.tile([P, n], F32, tag=f"c_{dram.name}")
            nc.sync.dma_start(t[:], dram.ap().rearrange("(a p) -> p a", p=P))
            return t

        pb = load_cols(pbias, KC)
        f1b = load_cols(fc1b, MFC1)
        f2b = load_cols(fc2b, KC)
        g1 = load_cols(g1d, KC)
        b1 = load_cols(b1d, KC)
        g2 = load_cols(g2d, KC)
        b2 = load_cols(b2d, KC)
        ind_sum_tiles = []
        for k in range(KC):
            tf = const.tile([P, P], F32, tag=f"indsf{k}")
            nc.sync.dma_start(tf[:], indsum[k * P:(k + 1) * P, :])
            t = const.tile([P, P], F32R, tag=f"inds{k}")
            nc.vector.tensor_copy(t[:], tf[:])
            ind_sum_tiles.append(t)
        ind_bTf = const.tile([3, D], F32)
        nc.sync.dma_start(ind_bTf[:], indbTd[:])
        ind_bT = const.tile([3, D], F32R)
        nc.vector.tensor_copy(ind_bT[:], ind_bTf[:])
        invlen = const.tile([3, 1], F32)
        nc.sync.dma_start(invlen[:], invlend[:])

        # proj weights first (pool outlives the attention-weight pool)
        wlate = ctx.enter_context(ExitStack())
        wp_pool = wlate.enter_context(tc.tile_pool(name="wp_pool", bufs=KP))
        wp8 = []
        for k2 in range(KP):
            t = wp_pool.tile([P, 2, D], F8, tag="wp8")
            nc.sync.dma_start(t[:], wp8d[k2 * P:(k2 + 1) * P, :])
            wp8.append(t)
        # attention weights (needed soon after LN1)
        wq_stage = ctx.enter_context(ExitStack())
        wq_pool = wq_stage.enter_context(tc.tile_pool(name="wqk", bufs=2 * KP))
        wqk8 = []
        wv8 = []
        for k2 in range(KP):
            t = wq_pool.tile([P, 2, 2 * D], F8, tag="wqk8")
            nc.sync.dma_start(t[:], wqk8d[k2 * P:(k2 + 1) * P, :])
            wqk8.append(t)
        for k2 in range(KP):
            t = wq_pool.tile([P, 2, D], F8, tag="wv8")
            nc.sync.dma_start(t[:], wv8d[k2 * P:(k2 + 1) * P, :])
            wv8.append(t)

        # ---- LN1 ----
        v_stage = ctx.enter_context(ExitStack())   # vT tiles, live thru attention
        v_pool = v_stage.enter_context(tc.tile_pool(name="v_pool", bufs=4))
        nx_stage = ctx.enter_context(ExitStack())  # normx pairs, live thru qkv/vT
        nx_pool = nx_stage.enter_context(tc.tile_pool(name="nx_pool", bufs=KP))
        nxp = [nx_pool.tile([P, 2, NT], F8, tag="nxp", name=f"nxp{i}")
           for i in range(KP)]
        for hi in range(2):
            hs = _halves()[hi]
            _layernorm_half(tc, hi,
                            lambda k, hs=hs: x_tiles[k][:, hs],
                            g1, b1, ind_sum_tiles, ind_bT,
                            invlen, eps_t,
                            lambda k, hs=hs: nxp[k // 2][:, k % 2, hs],
                            "ln1")

        if DEBUG:
            for k2 in range(KP):
                nc.sync.dma_start(dbg["nxp"][k2 * P:(k2 + 1) * P, :], nxp[k2][:])

        # ---- qkv (q,k) in fp8 DoubleRow; vT via PE (weights moving) ----
        y_stage = ctx.enter_context(ExitStack())
        y_pool = y_stage.enter_context(
            tc.tile_pool(name="y_pool", bufs=KP, side="right"))
        ytp = [y_pool.tile([P, 2, NT], F8, tag="ytp", name=f"ytp{i}")
           for i in range(KP)]
        qkv_stage = ctx.enter_context(ExitStack())
        q_pool = qkv_stage.enter_context(
            tc.tile_pool(name="q_pool", bufs=12, side="right"))

        psmm_stage = ctx.enter_context(ExitStack())
        psmm = psmm_stage.enter_context(
            tc.tile_pool(name="psmm", bufs=2, space="PSUM"))

        qt = []
        for m in range(2 * KC):  # q chunks 0..5, k chunks 6..11
            ps = psmm.tile([P, NT], F32, tag="mm")
            for hi, hs in enumerate(_halves()):
                for k2 in range(KP):
                    nc.tensor.matmul(ps[:, hs],
                                     wqk8[k2][:, :, m * P:(m + 1) * P],
                                     nxp[k2][:, :, hs],
                                     start=(k2 == 0), stop=(k2 == KP - 1),
                                     perf_mode=DR)
            t = q_pool.tile([P, NT], BF16, tag="qkt")
            nc.vector.tensor_copy(t[:], ps[:])
            qt.append(t)
            if DEBUG:
                nc.sync.dma_start(dbg["qt"][m * P:(m + 1) * P, :], t[:])

        # vT: out[tok, vdim] = sum_feat normx[feat, tok] * wv[feat, vdim]
        vtp = []
        for p2 in range(4):
            vt = v_pool.tile([P, 2, H * VG], F8, tag="vtp")
            # ones columns (col 64 of each 66-wide head group) for denominators
            for s in range(2):
                nc.vector.memset(
                    vt[:, s, :].rearrange("p (h c) -> p h c", c=VG)[:, :, HD:VG],
                    1.0)
            vtp.append(vt)
        with tc.tile_pool(name="vtps", bufs=2, space="PSUM") as vtps:
            for tc_i in range(8):
                vps = vtps.tile([P, D], F32, tag="vps")
                for cs in (slice(0, 512), slice(512, D)):
                    for k2 in range(KP):
                        nc.tensor.matmul(vps[:, cs],
                                         nxp[k2][:, :, tc_i * P:(tc_i + 1) * P],
                                         wv8[k2][:, :, cs],
                                         start=(k2 == 0), stop=(k2 == KP - 1),
                                         perf_mode=DR)
                dst = vtp[tc_i // 2][:, tc_i % 2, :].rearrange(
                    "p (h c) -> p h c", c=VG)[:, :, 0:HD]
                nc.vector.tensor_copy(
                    dst, vps[:].rearrange("p (h c) -> p h c", c=HD))

        if DEBUG:
            for p2 in range(4):
                nc.sync.dma_start(dbg["vtp"][p2 * P:(p2 + 1) * P, :], vtp[p2][:])

        # ---- attention, head by head ----
        with tc.tile_pool(name="e_pool", bufs=6) as e_pool, \
             tc.tile_pool(name="psot", bufs=2, space="PSUM") as psot, \
             tc.tile_pool(name="sm_pool", bufs=2) as sm_pool:

            def make_tail(h, ot):
                def tail():
                    dnm = sm_pool.tile([1, NT], F32, tag="dnm")
                    nc.vector.tensor_copy(dnm[:], ot[HD:HD + 1, :])
                    r = sm_pool.tile([1, NT], F32, tag="recip")
                    nc.vector.reciprocal_approx_fast(r[:], dnm[:])
                    if DEBUG:
                        nc.sync.dma_start(dbg["dnm"][h:h + 1, :], dnm[:])
                        nc.sync.dma_start(dbg["recip"][h:h + 1, :], r[:])
                    rbs = sm_pool.tile([HD, NT], F32, tag="rbs")
                    nc.gpsimd.partition_broadcast(rbs[:], r[:])
                    po2 = (h % 2) * HD
                    nc.vector.tensor_mul(
                        ytp[h // 4][po2:po2 + HD, (h // 2) % 2, :],
                        ot[0:HD, :], rbs[:])
                return tail

            pending_av = []
            pending_tails = []
            for h in range(H):
                j = h // 2
                po = (h % 2) * HD
                qsl = qt[j]
                ksl = qt[KC + j]
                ot = psot.tile([P, NT], F32, tag="ot")
                exs = [e_pool.tile([P, 2, NT], F8, tag="ex", name=f"ex{h}_{i}")
                       for i in range(4)]

                def av(p2, ot=ot, exs=exs, hh=h):
                    for hs in _halves():
                        nc.tensor.matmul(
                            ot[0:HD + 2, hs],
                            vtp[p2][:, :, hh * VG:hh * VG + HD + 2],
                            exs[p2][:, :, hs],
                            start=(p2 == 0), stop=(p2 == 3),
                            perf_mode=DR)

                for kc in range(8):
                    st = psmm.tile([P, NT], F32, tag="mm")
                    for hs in _halves():
                        nc.tensor.matmul(
                            st[:, hs],
                            ksl[po:po + HD, kc * P:(kc + 1) * P],
                            qsl[po:po + HD, hs],
                            start=True, stop=True)
                    nc.scalar.activation(exs[kc // 2][:, kc % 2, :],
                                         st[:], AF.Exp, scale=SCALE / 256.0)
                    if kc == 1 and pending_av:
                        pending_av.pop(0)()
                    if kc == 4 and pending_tails:
                        pending_tails.pop(0)()
                    if kc in (3, 5, 7):
                        av((kc - 3) // 2)
                pending_av.append(lambda av=av: av(3))
                pending_tails.append(make_tail(h, ot))
                if DEBUG and h == 0:
                    for p2 in range(4):
                        nc.sync.dma_start(dbg["ex0"][p2 * P:(p2 + 1) * P, :],
                                          exs[p2][:])
            while pending_av:
                pending_av.pop(0)()
            while pending_tails:
                pending_tails.pop(0)()
        if DEBUG:
            for k2 in range(KP):
                nc.sync.dma_start(dbg["ytp"][k2 * P:(k2 + 1) * P, :], ytp[k2][:])
        nx_stage.close()
        v_stage.close()
        wq_stage.close()
        qkv_stage.close()
        psmm_stage.close()

        # MLP weights: issue DMAs now (consumed ~20us later)
        wf1_pool = wlate.enter_context(tc.tile_pool(name="wf1_pool", bufs=KP))
        wf2_pool = wlate.enter_context(tc.tile_pool(name="wf2_pool", bufs=MP2))
        wf18 = []
        for k2 in range(KP):
            t = wf1_pool.tile([P, 2, HID], F8, tag="wf18")
            nc.sync.dma_start(t[:], wf18d[k2 * P:(k2 + 1) * P, :])
            wf18.append(t)
        wf28 = []
        for m2 in range(MP2):
            t = wf2_pool.tile([P, 2, D], F8, tag="wf28")
            nc.sync.dma_start(t[:], wf28d[m2 * P:(m2 + 1) * P, :])
            wf28.append(t)
        x1pool = ctx.enter_context(tc.tile_pool(name="x1pool", bufs=2 * KC))

        # ---- proj + residual (per token-half so LN2 starts early) ----
        x1 = {}
        with tc.tile_pool(name="pspj", bufs=2, space="PSUM") as pspj, \
             tc.tile_pool(name="ptp", bufs=2) as ptp:
            for hi, hs in enumerate(_halves()):
                for m in range(KC):
                    ps = pspj.tile([P, 512], F32, tag="pj")
                    for k2 in range(KP):
                        nc.tensor.matmul(ps[:],
                                         wp8[k2][:, :, m * P:(m + 1) * P],
                                         ytp[k2][:, :, hs],
                                         start=(k2 == 0), stop=(k2 == KP - 1),
                                         perf_mode=DR)
                    pt = ptp.tile([P, 512], F32, tag="pt")
                    nc.scalar.activation(pt[:], ps[:], AF.Identity,
                                         bias=pb[:, m:m + 1], scale=1.0 / 256.0)
                    xk = x1pool.tile([P, 512], F32R, tag="x1")
                    # x1 = proj_out + proj_b + x
                    nc.vector.tensor_tensor(
                        xk[:], pt[:], x_tiles[m][:, hs].bitcast(F32), ALU.add)
                    x1[(m, hi)] = xk
                    if DEBUG:
                        nc.sync.dma_start(dbg["x1"][m * P:(m + 1) * P, hs], xk[:])
        y_stage.close()

        # ---- LN2 + MLP, pipelined per token-half ----
        nx2_stage = ctx.enter_context(ExitStack())
        nx2_pool = nx2_stage.enter_context(tc.tile_pool(name="nx2_pool", bufs=KP))
        nx2p = [nx2_pool.tile([P, 2, NT], F8, tag="nx2p", name=f"nx2p{i}")
            for i in range(KP)]
        h_stage = ctx.enter_context(ExitStack())
        h_pool = h_stage.enter_context(
            tc.tile_pool(name="h_pool", bufs=MP2, side="right"))
        h8 = [h_pool.tile([P, 2, NT], F8, tag="h8", name=f"h8_{i}")
          for i in range(MP2)]

        with tc.tile_pool(name="psmlp", bufs=4, space="PSUM") as psmlp, \
             tc.tile_pool(name="o_pool", bufs=2) as o_pool:
            for hi in range(2):
                hs = _halves()[hi]
                _layernorm_half(tc, hi, lambda k, hi=hi: x1[(k, hi)][:],
                                g2, b2, ind_sum_tiles, ind_bT,
                                invlen, eps_t,
                                lambda k, hs=hs: nx2p[k // 2][:, k % 2, hs],
                                "ln2")
                # fc1 + gelu for this half
                for m in range(MFC1):
                    ps = psmlp.tile([P, 512], F32, tag="mlp")
                    for k2 in range(KP):
                        nc.tensor.matmul(ps[:],
                                         wf18[k2][:, :, m * P:(m + 1) * P],
                                         nx2p[k2][:, :, hs],
                                         start=(k2 == 0), stop=(k2 == KP - 1),
                                         perf_mode=DR)
                    nc.scalar.activation(h8[m // 2][:, m % 2, hs], ps[:],
                                         AF.Gelu, bias=f1b[:, m:m + 1],
                                         scale=1.0 / 16.0)
                # fc2 + residual for this half
                for m in range(KC):
                    ps = psmlp.tile([P, 512], F32, tag="mlp")
                    for m2 in range(MP2):
                        nc.tensor.matmul(ps[:],
                                         wf28[m2][:, :, m * P:(m + 1) * P],
                                         h8[m2][:, :, hs],
                                         start=(m2 == 0), stop=(m2 == MP2 - 1),
                                         perf_mode=DR)
                    ft = o_pool.tile([P, 512], F32, tag="ft")
                    nc.scalar.activation(ft[:], ps[:], AF.Identity,
                                         bias=f2b[:, m:m + 1], scale=1.0 / 16.0)
                    ok = o_pool.tile([P, 512], F32, tag="o")
                    # out = fc2_out + fc2_b + x1
                    nc.vector.tensor_tensor(
                        ok[:], ft[:], x1[(m, hi)][:].bitcast(F32), ALU.add)
                    nc.sync.dma_start(outT[m * P:(m + 1) * P, hs], ok[:])

    nc.compile()
    return nc


_NC = None


def _get_nc():
    global _NC
    if _NC is None:
        _NC = build()
    return _NC


def _pair_rows(wT):
    """[K, F] fp32 row-major -> [K//2, 2*F] fp8 with 128-row slab pairs:
    out[p*128+i, s*F+m] = wT[256*p + 128*s + i, m]."""
    K, F = wT.shape
    return np.ascontiguousarray(
        wT.reshape(K // 256, 2, 128, F).transpose(0, 2, 1, 3).reshape(K // 2, 2 * F)
    ).astype(ml_dtypes.float8_e4m3)


def _prep_inputs(inputs):
    f32 = np.float32
    g = {k: np.asarray(v) for k, v in inputs.items()}
    qkvT = np.ascontiguousarray(g["qkv_w"].astype(f32).T)  # [D, 3D]
    shared = {
        "wqk8": _pair_rows(16.0 * qkvT[:, 0:2 * D]),
        "wv8": _pair_rows(16.0 * qkvT[:, 2 * D:3 * D]),
        "wp8": _pair_rows(16.0 * np.ascontiguousarray(g["proj_w"].astype(f32).T)),
        "wf18": _pair_rows(16.0 * np.ascontiguousarray(g["fc1_w"].astype(f32).T)),
        "wf28": _pair_rows(16.0 * np.ascontiguousarray(g["fc2_w"].astype(f32).T)),
        "pbias": np.ascontiguousarray(g["proj_b"], dtype=f32),
        "fc1b": np.ascontiguousarray(g["fc1_b"], dtype=f32),
        "fc2b": np.ascontiguousarray(g["fc2_b"], dtype=f32),
        "g1": np.concatenate([g["ln1a_g"], g["ln1b_g"], g["ln1c_g"]]).astype(f32),
        "b1": np.concatenate([g["ln1a_b"], g["ln1b_b"], g["ln1c_b"]]).astype(f32),
        "g2": np.concatenate([g["ln2a_g"], g["ln2b_g"], g["ln2c_g"]]).astype(f32),
        "b2": np.concatenate([g["ln2a_b"], g["ln2b_b"], g["ln2c_b"]]).astype(f32),
    }
    ind = np.zeros((D, 128), dtype=f32)
    ind[0:S1, 0] = 1.0
    ind[S1:S2, 1] = 1.0
    ind[S2:D, 2] = 1.0
    shared["indsum"] = ind
    shared["indbT"] = np.ascontiguousarray(ind[:, 0:3].T)
    shared["invlen"] = np.array([[1.0 / S1], [1.0 / (S2 - S1)],
                                 [1.0 / (D - S2)]], dtype=f32)
    x = np.asarray(g["x"], dtype=f32)
    in_maps = []
    for b in range(B):
        m = dict(shared)
        m["xT"] = np.ascontiguousarray(x[b].T)
        in_maps.append(m)
    return in_maps


def run(inputs, trace=False):
    nc = _get_nc()
    in_maps = _prep_inputs(inputs)
    res = run_bass_kernel_spmd(nc, in_maps, core_ids=list(range(B)),
                               trace=trace)
    out = np.stack([np.ascontiguousarray(res.results[b]["outT"].T)
                    for b in range(B)]).astype(np.float32)
    return out, res


def kernel(**inputs):
    out, _ = run(inputs, trace=False)
    return out
